# revision 34
# baseline (speedup 1.0000x reference)
import math
import numpy as np

SR, SEG, NH, BASE_F = 48000, 960, 8, 220.0
N, C, Lf = 32, 256, 250
Lw = Lf * SEG
NCORES = 8
NPC = N // NCORES   # 4 samples per core
HP = Lf // 2
MAGIC = 12582912.0

_cache = {}


def _bf(v):
    import ml_dtypes
    return np.asarray(v, np.float32).astype(ml_dtypes.bfloat16)


def _consts():
    s = np.arange(SEG, dtype=np.float64)
    delta = (s + 0.5) / SEG - 0.5
    lo = s < SEG // 2
    a_s = np.where(lo, -delta, 0.0)
    b_s = np.where(lo, 1 + delta, 1 - delta)
    d_s = np.where(lo, 0.0, delta)
    A = (np.cumsum(a_s) / SR).astype(np.float32)
    D = (np.cumsum(d_s) / SR).astype(np.float32)
    R = (np.cumsum(a_s + b_s + d_s) / SR).astype(np.float32)
    R_hi = _bf(R).astype(np.float32)
    R_lo = R - R_hi
    ucoef = np.stack([R_hi, R_hi, A, D, R_lo, np.ones(SEG, np.float32),
                      np.ones(SEG, np.float32)])
    rhs = np.zeros((128, SEG), np.float32)
    rhs[32:39] = ucoef
    rhs[39] = MAGIC
    rhs[40] = -MAGIC
    rhs[41:48] = -ucoef
    coef = np.stack([a_s, b_s, d_s]).astype(np.float32)  # [3, 960]
    for sh in range(3):
        for h in range(NH):
            rhs[64 + sh * 8 + h] = -coef[sh] / NH
    return _bf(rhs)


def _build():
    import concourse.bacc as bacc
    import concourse.mybir as mybir
    import concourse.tile as tile
    from concourse.ap import AP
    from contextlib import ExitStack

    f32 = mybir.dt.float32
    bf16 = mybir.dt.bfloat16
    f16 = mybir.dt.float16
    AF = mybir.ActivationFunctionType
    AL = mybir.AluOpType
    LN2 = float(np.log(2.0))
    TWO_PI = float(2.0 * np.pi)
    c1 = 120.0 / SR
    c2 = 720.0 / SR

    nc = bacc.Bacc("TRN2", target_bir_lowering=False, debug=False)
    x_d = nc.dram_tensor("xr", [128, NPC * 500], f32, kind="ExternalInput")
    xb_d = nc.dram_tensor("xrb", [128, NPC * 500], bf16, kind="ExternalInput")
    cst_d = nc.dram_tensor("cst", [128, 8], f32, kind="ExternalInput")
    rhs_d = nc.dram_tensor("rhsb", [128, SEG + 16], bf16, kind="ExternalInput")
    out_d = nc.dram_tensor("wave", [NPC, Lf, SEG], f16, kind="ExternalOutput")
    on_d = nc.dram_tensor("ones2", [2, NPC * 250], bf16, kind="ExternalInput")
    stg_d = nc.dram_tensor("stgd", [NPC, 1280], bf16, kind="Internal")
    sb_d = nc.dram_tensor("stgb", [NPC, 512], bf16, kind="Internal")
    mp_d = nc.dram_tensor("mpd", [NH, 1008], bf16, kind="Internal")

    with tile.TileContext(nc) as tc, ExitStack() as ctx:
        const = ctx.enter_context(tc.tile_pool(name="const", bufs=1))
        stage = ctx.enter_context(tc.tile_pool(name="stage", bufs=1))
        snp = ctx.enter_context(tc.tile_pool(name="snp", bufs=2))
        wvp = ctx.enter_context(tc.tile_pool(name="wvp", bufs=2))
        ps_c = ctx.enter_context(tc.tile_pool(name="ps_c", bufs=1, space="PSUM"))
        ps_v = ctx.enter_context(tc.tile_pool(name="ps_v", bufs=2, space="PSUM"))
        ps_m = ctx.enter_context(tc.tile_pool(name="ps_m", bufs=3, space="PSUM"))

        cst = const.tile([128, 8], f32)
        nc.sync.dma_start(cst[:], cst_d[:])
        rhsb = const.tile([128, SEG + 16], bf16)
        nc.gpsimd.dma_start(rhsb[:], rhs_d[:])
        xall = const.tile([128, NPC * 500], f32)
        xbf = const.tile([128, NPC * 500], bf16)
        qs = [nc.sync, nc.gpsimd, nc.scalar]
        qi = 0
        for co in range(2):
            for ph in range(4):
                p0, p1 = ph * 32, ph * 32 + 32
                xv = xall[p0:p1, :].rearrange(
                    "p (c q) -> p c q", c=2)[:, :, co * 500:(co + 1) * 500]
                xs = x_d[p0:p1, :].rearrange(
                    "p (c q) -> p c q", c=2)[:, :, co * 500:(co + 1) * 500]
                qs[qi % 3].dma_start(xv, xs)
                qi += 1
            for ph in range(4):
                p0, p1 = ph * 32, ph * 32 + 32
                xbv = xbf[p0:p1, :].rearrange(
                    "p (c q) -> p c q", c=2)[:, :, co * 500:(co + 1) * 500]
                xbs = xb_d[p0:p1, :].rearrange(
                    "p (c q) -> p c q", c=2)[:, :, co * 500:(co + 1) * 500]
                qs[qi % 3].dma_start(xbv, xbs)
                qi += 1
        zbias = const.tile([125, 1], f32)
        nc.gpsimd.memset(zbias[:], 0.0)
        lhs = const.tile([128, NPC * 250], bf16)
        nc.scalar.dma_start(lhs[39:41, :], on_d[:])

        # ================= two-batch pipelined body =================
        gtmp = stage.tile([1, 1000], f32)
        zz = stage.tile([NH, 1008], f32)
        ccs = []
        for bb in range(2):
            cc = ps_c.tile([128, 512], f32)
            ccs.append(cc)
            r0 = bb * 500
            nc.tensor.matmul(cc[32:33, 0:500], cst[:, 0:1],
                             xall[:, r0:r0 + 500], start=True, stop=False,
                             tile_position=(0, 32))
            nc.tensor.matmul(cc[32:33, 0:500], cst[:, 1:2],
                             xall[:, 1000 + r0:1000 + r0 + 500],
                             start=False, stop=True, tile_position=(0, 32))
            nc.tensor.matmul(cc[0:8, 0:500], rhsb[:, 960 + 0:960 + 8],
                             xbf[:, r0:r0 + 500], start=True, stop=False)
            nc.tensor.matmul(cc[0:8, 0:500], rhsb[:, 968:976],
                             xbf[:, 1000 + r0:1000 + r0 + 500],
                             start=False, stop=True)

        # all ACT exp work up front (single exp-table residency)
        gpads = []
        magps = []
        for bb in range(2):
            cc = ccs[bb]
            nc.scalar.activation(gtmp[0:1, bb * 500:bb * 500 + 500],
                                 cc[32:33, 0:500], AF.Exp,
                                 bias=cst[0:1, 4:5], scale=LN2)
            gpad2 = stage.tile([2, Lf + 2], f32)
            gpads.append(gpad2)
            gsrc = AP(tensor=gtmp.tensor, offset=gtmp[:].offset + bb * 500,
                      ap=[[1000, 1], [250, 2], [1, 250]])
            nc.sync.dma_start(gpad2[0:2, 1:251], gsrc)
            zi = AP(tensor=cc[:].tensor, offset=cc[:].offset,
                    ap=[[512, 8], [250, 2], [1, 250]])
            zo = AP(tensor=zz.tensor, offset=zz[:].offset + 1 + bb * 504,
                    ap=[[1008, 8], [252, 2], [1, 250]])
            nc.vector.tensor_scalar(zo, zi, cst[0:8, 2:3], 6.0, AL.add, AL.min)
            magp2 = stage.tile([NH, 504], bf16)
            magps.append(magp2)
            mo = AP(tensor=magp2.tensor, offset=magp2[:].offset + 1,
                    ap=[[504, 8], [252, 2], [1, 250]])
            zo2 = AP(tensor=zz.tensor, offset=zz[:].offset + 1 + bb * 504,
                     ap=[[1008, 8], [252, 2], [1, 250]])
            nc.scalar.activation(mo, zo2, AF.Exp)
            eL = AP(tensor=magp2.tensor, offset=magp2[:].offset,
                    ap=[[504, 8], [252, 2], [1, 1]])
            eLs = AP(tensor=magp2.tensor, offset=magp2[:].offset + 1,
                     ap=[[504, 8], [252, 2], [1, 1]])
            nc.gpsimd.tensor_copy(eL, eLs)
            eR = AP(tensor=magp2.tensor, offset=magp2[:].offset + 251,
                    ap=[[504, 8], [252, 2], [1, 1]])
            eRs = AP(tensor=magp2.tensor, offset=magp2[:].offset + 250,
                     ap=[[504, 8], [252, 2], [1, 1]])
            nc.gpsimd.tensor_copy(eR, eRs)
            nc.gpsimd.dma_start(mp_d[:, bb * 504:(bb + 1) * 504], magp2[:])
            for m in range(2):
                nn = 2 * bb + m
                msrc = AP(tensor=mp_d[:].tensor, offset=nn * 252,
                          ap=[[1, 3], [1008, 8], [1, 250]])
                nc.gpsimd.dma_start(lhs[64:88, nn * 250:(nn + 1) * 250], msrc)

        sindum = stage.tile([1, 1], f32)
        nc.scalar.activation(sindum[:], magps[1][0:1, 0:1], AF.Sin,
                             bias=zbias[0:1, 0:1], scale=TWO_PI)

        def stage_batch(bb):
            gpad2 = gpads[bb]
            nc.vector.tensor_copy(gpad2[:, 0:1], gpad2[:, 1:2])
            nc.vector.tensor_copy(gpad2[:, Lf + 1:Lf + 2], gpad2[:, Lf:Lf + 1])
            stg2 = stage.tile([2, 1280], bf16)
            nc.gpsimd.tensor_copy(stg2[0:2, 0:252], gpad2[:])
            nc.vector.tensor_tensor(stg2[0:2, 256:508], gpad2[:],
                                    stg2[0:2, 0:252], AL.subtract)
            nc.gpsimd.tensor_tensor(stg2[0:2, 513:763], gpad2[:, 0:250],
                                    gpad2[:, 1:251], AL.subtract)
            nc.gpsimd.tensor_tensor(stg2[0:2, 769:1019], gpad2[:, 2:252],
                                    gpad2[:, 1:251], AL.subtract)
            nc.gpsimd.tensor_copy(stg2[0:2, 1024:1276], stg2[0:2, 0:252])
            nc.sync.dma_start(stg_d[2 * bb:2 * bb + 2, :], stg2[:])
            u5 = AP(tensor=stg_d[:].tensor, offset=1 + bb * 2560,
                    ap=[[256, 5], [1280, 2], [1, 250]])
            nc.sync.dma_start(lhs[32:37, bb * 500:(bb + 1) * 500], u5)
            u5b = AP(tensor=stg_d[:].tensor, offset=1 + bb * 2560,
                     ap=[[256, 5], [1280, 2], [1, 250]])
            nc.sync.dma_start(lhs[41:46, bb * 500:(bb + 1) * 500], u5b)
            # prefix chain
            t1 = stage.tile([2, Lf], f32)
            nc.vector.tensor_tensor(t1[:], gpad2[:, 0:250], gpad2[:, 2:252],
                                    AL.add)
            xg = stage.tile([2, Lf], f32)
            nc.vector.tensor_scalar(xg[:], gpad2[:, 1:251], c2, None, AL.mult)
            srow = stage.tile([2, Lf], f32)
            nc.vector.scalar_tensor_tensor(srow[:], t1[:], c1, xg[:],
                                           AL.mult, AL.add)
            rs_ = stage.tile([2, Lf], f32)
            nc.vector.tensor_scalar(rs_[:], srow[:], MAGIC, MAGIC,
                                    AL.add, AL.subtract)
            sf_ = stage.tile([2, Lf], f32)
            nc.vector.tensor_tensor(sf_[:], srow[:], rs_[:], AL.subtract)
            pinc = stage.tile([2, Lf], f32)
            nc.vector.tensor_tensor_scan(pinc[:], sf_[:], sf_[:], 0.0,
                                         AL.add, AL.bypass)
            b0 = stage.tile([2, Lf], f32)
            nc.vector.tensor_tensor(b0[:], pinc[:], sf_[:], AL.subtract)
            b1 = stage.tile([2, Lf], f32)
            nc.vector.tensor_scalar(b1[:], b0[:],
                                    cst[0:2, 3 + 2 * bb:4 + 2 * bb],
                                    None, AL.add)
            rb_ = stage.tile([2, Lf], f32)
            nc.vector.tensor_scalar(rb_[:], b1[:], MAGIC, MAGIC,
                                    AL.add, AL.subtract)
            b2 = stage.tile([2, Lf], f32)
            nc.vector.scalar_tensor_tensor(b2[:], b1[:], 16.0, rb_[:],
                                           AL.add, AL.subtract)
            stgB = stage.tile([2, 512], bf16)
            nc.vector.tensor_copy(stgB[0:2, 0:250], b2[:, 0:250])
            nc.vector.tensor_tensor(stgB[0:2, 256:506], b2[:, 0:250],
                                    stgB[0:2, 0:250], AL.subtract)
            nc.sync.dma_start(sb_d[2 * bb:2 * bb + 2, :], stgB[:])
            bsrc = AP(tensor=sb_d[:].tensor, offset=bb * 1024,
                      ap=[[256, 2], [512, 2], [1, 250]])
            nc.sync.dma_start(lhs[37:39, bb * 500:(bb + 1) * 500], bsrc)
            bsrc2 = AP(tensor=sb_d[:].tensor, offset=bb * 1024,
                       ap=[[256, 2], [512, 2], [1, 250]])
            nc.scalar.dma_start(lhs[46:48, bb * 500:(bb + 1) * 500], bsrc2)

        def main_batch(bb):
            for n in (2 * bb, 2 * bb + 1):
                sn = snp.tile([125, 1920], f16)
                wv = wvp.tile([125, 1920], f16)
                for h in range(2):
                    col0 = n * 250 + h * 125
                    pv = ps_v.tile([125, 1024], f32)
                    pms = []
                    for cchunk in range(2):
                        s0 = cchunk * 480
                        nc.tensor.matmul(pv[:, cchunk * 512:cchunk * 512 + 480],
                                         lhs[32:48, col0:col0 + 125],
                                         rhsb[32:48, s0:s0 + 480],
                                         start=True, stop=True,
                                         tile_position=(32, 0))
                        pm = ps_m.tile([125, 512], f32)
                        nc.tensor.matmul(pm[:, 0:480],
                                         lhs[64:88, col0:col0 + 125],
                                         rhsb[64:88, s0:s0 + 480],
                                         start=True, stop=True,
                                         tile_position=(64, 0))
                        pms.append(pm)
                    pvv = pv[:].rearrange("p (b c) -> p b c", b=2)[:, :, 0:480]
                    snv = sn[:, h * 960:(h + 1) * 960].rearrange(
                        "p (b c) -> p b c", b=2)
                    nc.scalar.activation(snv, pvv, AF.Sin, bias=zbias[:, 0:1],
                                         scale=TWO_PI)
                    for cchunk in range(2):
                        q = h * 2 + cchunk
                        nc.vector.tensor_tensor(wv[:, q * 480:(q + 1) * 480],
                                                pms[cchunk][:, 0:480],
                                                sn[:, q * 480:(q + 1) * 480],
                                                AL.mult)
                for h in range(2):
                    oq = nc.sync if h == 0 else nc.gpsimd
                    oq.dma_start(out_d[n, h * 125:(h + 1) * 125, :],
                                 wv[:, h * 960:(h + 1) * 960])

        stage_batch(0)
        main_batch(0)
        stage_batch(1)
        main_batch(1)

    nc.compile()
    return nc


def _make_in_maps(inputs):
    x, phi, w_mag, b_mag, w_oct, b_oct = (inputs[k] for k in (
        "x", "phi", "w_mag", "b_mag", "w_oct", "b_oct"))
    rhs_base = _consts()  # [128, 960] bf16
    rhsb = np.zeros((128, SEG + 16), np.float32)
    rhsb[:, 0:SEG] = rhs_base.astype(np.float32)
    wm = w_mag[:, :, 0].astype(np.float32)  # [8, 256]
    for cc in range(2):
        rhsb[:, SEG + cc * 8:SEG + cc * 8 + 8] = wm[:, cc * 128:(cc + 1) * 128].T
    rhsb = _bf(rhsb)
    in_maps = []
    for c in range(NCORES):
        xs = np.ascontiguousarray(x[c * NPC:(c + 1) * NPC]).astype(np.float32)
        # cols: cc*1000 + n*250 + l
        xr = xs.reshape(NPC, 2, 128, 250).transpose(2, 1, 0, 3).reshape(128, 2000)
        xr = np.ascontiguousarray(xr)
        cst = np.zeros((128, 8), np.float32)
        cst[:, 0] = w_oct[0, 0:128, 0]
        cst[:, 1] = w_oct[0, 128:256, 0]
        cst[0:8, 2] = b_mag
        cst[0:2, 3] = phi[c * NPC:c * NPC + 2, 0, 0]
        cst[0:2, 5] = phi[c * NPC + 2:c * NPC + 4, 0, 0]
        cst[0, 4] = math.log(220.0) + math.log(2.0) * float(b_oct[0])
        in_maps.append(dict(xr=xr, xrb=_bf(xr), cst=cst, rhsb=rhsb,
                            ones2=_bf(np.ones((2, NPC * 250), np.float32))))
    return in_maps


def kernel(x, phi, w_mag, b_mag, w_oct, b_oct):
    from concourse.bass_utils import run_bass_kernel_spmd

    if "nc" not in _cache:
        _cache["nc"] = _build()
    nc = _cache["nc"]

    in_maps = _make_in_maps(dict(x=x, phi=phi, w_mag=w_mag, b_mag=b_mag,
                                 w_oct=w_oct, b_oct=b_oct))
    res = run_bass_kernel_spmd(nc, in_maps, core_ids=list(range(NCORES)))
    waves = [res.results[c]["wave"].astype(np.float32).reshape(NPC, 1, Lw)
             for c in range(NCORES)]
    return np.concatenate(waves, axis=0)


# revision 36
# speedup vs baseline: 1.0828x; 1.0828x over previous
import math
import numpy as np

SR, SEG, NH, BASE_F = 48000, 960, 8, 220.0
N, C, Lf = 32, 256, 250
Lw = Lf * SEG
NCORES = 8
NPC = N // NCORES   # 4 samples per core
HP = Lf // 2
MAGIC = 12582912.0

_cache = {}


def _bf(v):
    import ml_dtypes
    return np.asarray(v, np.float32).astype(ml_dtypes.bfloat16)


def _consts():
    s = np.arange(SEG, dtype=np.float64)
    delta = (s + 0.5) / SEG - 0.5
    lo = s < SEG // 2
    a_s = np.where(lo, -delta, 0.0)
    b_s = np.where(lo, 1 + delta, 1 - delta)
    d_s = np.where(lo, 0.0, delta)
    A = (np.cumsum(a_s) / SR).astype(np.float32)
    D = (np.cumsum(d_s) / SR).astype(np.float32)
    R = (np.cumsum(a_s + b_s + d_s) / SR).astype(np.float32)
    R_hi = _bf(R).astype(np.float32)
    R_lo = R - R_hi
    ucoef = np.stack([R_hi, R_hi, A, D, R_lo, np.ones(SEG, np.float32),
                      np.ones(SEG, np.float32)])
    rhs = np.zeros((128, SEG), np.float32)
    rhs[32:39] = ucoef
    rhs[39] = MAGIC
    rhs[40] = -MAGIC
    rhs[41:48] = -ucoef
    coef = np.stack([a_s, b_s, d_s]).astype(np.float32)  # [3, 960]
    for sh in range(3):
        for h in range(NH):
            rhs[64 + sh * 8 + h] = -coef[sh] / NH
    return _bf(rhs)


def _build():
    import concourse.bacc as bacc
    import concourse.mybir as mybir
    import concourse.tile as tile
    from concourse.ap import AP
    from contextlib import ExitStack

    f32 = mybir.dt.float32
    bf16 = mybir.dt.bfloat16
    f16 = mybir.dt.float16
    AF = mybir.ActivationFunctionType
    AL = mybir.AluOpType
    LN2 = float(np.log(2.0))
    TWO_PI = float(2.0 * np.pi)
    c1 = 120.0 / SR
    c2 = 720.0 / SR

    nc = bacc.Bacc("TRN2", target_bir_lowering=False, debug=False)
    x_d = nc.dram_tensor("xr", [128, NPC * 500], f32, kind="ExternalInput")
    xb_d = nc.dram_tensor("xrb", [128, NPC * 500], bf16, kind="ExternalInput")
    cst_d = nc.dram_tensor("cst", [128, 8], f32, kind="ExternalInput")
    rhs_d = nc.dram_tensor("rhsb", [128, SEG + 16], bf16, kind="ExternalInput")
    out_d = nc.dram_tensor("wave", [NPC, Lf, SEG], f16, kind="ExternalOutput")
    on_d = nc.dram_tensor("ones2", [2, NPC * 250], bf16, kind="ExternalInput")
    stg_d = nc.dram_tensor("stgd", [NPC, 1280], bf16, kind="Internal")
    sb_d = nc.dram_tensor("stgb", [NPC, 512], bf16, kind="Internal")
    mp_d = nc.dram_tensor("mpd", [NH, 1008], bf16, kind="Internal")

    with tile.TileContext(nc) as tc, ExitStack() as ctx:
        const = ctx.enter_context(tc.tile_pool(name="const", bufs=1))
        stage = ctx.enter_context(tc.tile_pool(name="stage", bufs=1))
        snp = ctx.enter_context(tc.tile_pool(name="snp", bufs=3))
        wvp = ctx.enter_context(tc.tile_pool(name="wvp", bufs=3))
        ps_c = ctx.enter_context(tc.tile_pool(name="ps_c", bufs=1, space="PSUM"))
        ps_v = ctx.enter_context(tc.tile_pool(name="ps_v", bufs=2, space="PSUM"))
        ps_m = ctx.enter_context(tc.tile_pool(name="ps_m", bufs=3, space="PSUM"))

        cst = const.tile([128, 8], f32)
        nc.sync.dma_start(cst[:], cst_d[:])
        rhsb = const.tile([128, SEG + 16], bf16)
        nc.gpsimd.dma_start(rhsb[:], rhs_d[:])
        xall = const.tile([128, NPC * 500], f32)
        xbf = const.tile([128, NPC * 500], bf16)
        qs = [nc.sync, nc.gpsimd, nc.scalar]
        qi = 0
        for co in range(2):
            for ph in range(4):
                p0, p1 = ph * 32, ph * 32 + 32
                xv = xall[p0:p1, :].rearrange(
                    "p (c q) -> p c q", c=2)[:, :, co * 500:(co + 1) * 500]
                xs = x_d[p0:p1, :].rearrange(
                    "p (c q) -> p c q", c=2)[:, :, co * 500:(co + 1) * 500]
                qs[qi % 3].dma_start(xv, xs)
                qi += 1
            for ph in range(4):
                p0, p1 = ph * 32, ph * 32 + 32
                xbv = xbf[p0:p1, :].rearrange(
                    "p (c q) -> p c q", c=2)[:, :, co * 500:(co + 1) * 500]
                xbs = xb_d[p0:p1, :].rearrange(
                    "p (c q) -> p c q", c=2)[:, :, co * 500:(co + 1) * 500]
                qs[qi % 3].dma_start(xbv, xbs)
                qi += 1
        zbias = const.tile([125, 1], f32)
        nc.gpsimd.memset(zbias[:], 0.0)
        lhs = const.tile([128, NPC * 250], bf16)
        nc.scalar.dma_start(lhs[39:41, :], on_d[:])

        # ================= two-batch pipelined body =================
        gtmp = stage.tile([1, 1000], f32)
        zz = stage.tile([NH, 1008], f32)
        ccs = []
        for bb in range(2):
            cc = ps_c.tile([128, 512], f32)
            ccs.append(cc)
            r0 = bb * 500
            nc.tensor.matmul(cc[32:33, 0:500], cst[:, 0:1],
                             xall[:, r0:r0 + 500], start=True, stop=False,
                             tile_position=(0, 32))
            nc.tensor.matmul(cc[32:33, 0:500], cst[:, 1:2],
                             xall[:, 1000 + r0:1000 + r0 + 500],
                             start=False, stop=True, tile_position=(0, 32))
            nc.tensor.matmul(cc[0:8, 0:500], rhsb[:, 960 + 0:960 + 8],
                             xbf[:, r0:r0 + 500], start=True, stop=False)
            nc.tensor.matmul(cc[0:8, 0:500], rhsb[:, 968:976],
                             xbf[:, 1000 + r0:1000 + r0 + 500],
                             start=False, stop=True)

        # all ACT exp work up front (single exp-table residency)
        gpads = []
        magps = []
        for bb in range(2):
            cc = ccs[bb]
            nc.scalar.activation(gtmp[0:1, bb * 500:bb * 500 + 500],
                                 cc[32:33, 0:500], AF.Exp,
                                 bias=cst[0:1, 4:5], scale=LN2)
            gpad2 = stage.tile([2, Lf + 2], f32)
            gpads.append(gpad2)
            gsrc = AP(tensor=gtmp.tensor, offset=gtmp[:].offset + bb * 500,
                      ap=[[1000, 1], [250, 2], [1, 250]])
            nc.sync.dma_start(gpad2[0:2, 1:251], gsrc)
            zi = AP(tensor=cc[:].tensor, offset=cc[:].offset,
                    ap=[[512, 8], [250, 2], [1, 250]])
            zo = AP(tensor=zz.tensor, offset=zz[:].offset + 1 + bb * 504,
                    ap=[[1008, 8], [252, 2], [1, 250]])
            nc.vector.tensor_scalar(zo, zi, cst[0:8, 2:3], 6.0, AL.add, AL.min)
            magp2 = stage.tile([NH, 504], bf16)
            magps.append(magp2)
            mo = AP(tensor=magp2.tensor, offset=magp2[:].offset + 1,
                    ap=[[504, 8], [252, 2], [1, 250]])
            zo2 = AP(tensor=zz.tensor, offset=zz[:].offset + 1 + bb * 504,
                     ap=[[1008, 8], [252, 2], [1, 250]])
            nc.scalar.activation(mo, zo2, AF.Exp)
            eL = AP(tensor=magp2.tensor, offset=magp2[:].offset,
                    ap=[[504, 8], [252, 2], [1, 1]])
            eLs = AP(tensor=magp2.tensor, offset=magp2[:].offset + 1,
                     ap=[[504, 8], [252, 2], [1, 1]])
            nc.gpsimd.tensor_copy(eL, eLs)
            eR = AP(tensor=magp2.tensor, offset=magp2[:].offset + 251,
                    ap=[[504, 8], [252, 2], [1, 1]])
            eRs = AP(tensor=magp2.tensor, offset=magp2[:].offset + 250,
                     ap=[[504, 8], [252, 2], [1, 1]])
            nc.gpsimd.tensor_copy(eR, eRs)
            nc.gpsimd.dma_start(mp_d[:, bb * 504:(bb + 1) * 504], magp2[:])
            for m in range(2):
                nn = 2 * bb + m
                msrc = AP(tensor=mp_d[:].tensor, offset=nn * 252,
                          ap=[[1, 3], [1008, 8], [1, 250]])
                nc.gpsimd.dma_start(lhs[64:88, nn * 250:(nn + 1) * 250], msrc)

        sindum = stage.tile([1, 1], f32)
        nc.scalar.activation(sindum[:], magps[1][0:1, 0:1], AF.Sin,
                             bias=zbias[0:1, 0:1], scale=TWO_PI)

        def stage_batch(bb):
            gpad2 = gpads[bb]
            nc.vector.tensor_copy(gpad2[:, 0:1], gpad2[:, 1:2])
            nc.vector.tensor_copy(gpad2[:, Lf + 1:Lf + 2], gpad2[:, Lf:Lf + 1])
            stg2 = stage.tile([2, 1280], bf16)
            nc.gpsimd.tensor_copy(stg2[0:2, 0:252], gpad2[:])
            nc.vector.tensor_tensor(stg2[0:2, 256:508], gpad2[:],
                                    stg2[0:2, 0:252], AL.subtract)
            nc.gpsimd.tensor_tensor(stg2[0:2, 513:763], gpad2[:, 0:250],
                                    gpad2[:, 1:251], AL.subtract)
            nc.gpsimd.tensor_tensor(stg2[0:2, 769:1019], gpad2[:, 2:252],
                                    gpad2[:, 1:251], AL.subtract)
            nc.gpsimd.tensor_copy(stg2[0:2, 1024:1276], stg2[0:2, 0:252])
            nc.sync.dma_start(stg_d[2 * bb:2 * bb + 2, :], stg2[:])
            u5 = AP(tensor=stg_d[:].tensor, offset=1 + bb * 2560,
                    ap=[[256, 5], [1280, 2], [1, 250]])
            nc.sync.dma_start(lhs[32:37, bb * 500:(bb + 1) * 500], u5)
            u5b = AP(tensor=stg_d[:].tensor, offset=1 + bb * 2560,
                     ap=[[256, 5], [1280, 2], [1, 250]])
            nc.sync.dma_start(lhs[41:46, bb * 500:(bb + 1) * 500], u5b)
            # prefix chain
            t1 = stage.tile([2, Lf], f32)
            nc.vector.tensor_tensor(t1[:], gpad2[:, 0:250], gpad2[:, 2:252],
                                    AL.add)
            xg = stage.tile([2, Lf], f32)
            nc.vector.tensor_scalar(xg[:], gpad2[:, 1:251], c2, None, AL.mult)
            srow = stage.tile([2, Lf], f32)
            nc.vector.scalar_tensor_tensor(srow[:], t1[:], c1, xg[:],
                                           AL.mult, AL.add)
            rs_ = stage.tile([2, Lf], f32)
            nc.vector.tensor_scalar(rs_[:], srow[:], MAGIC, MAGIC,
                                    AL.add, AL.subtract)
            sf_ = stage.tile([2, Lf], f32)
            nc.vector.tensor_tensor(sf_[:], srow[:], rs_[:], AL.subtract)
            pinc = stage.tile([2, Lf], f32)
            nc.vector.tensor_tensor_scan(pinc[:], sf_[:], sf_[:], 0.0,
                                         AL.add, AL.bypass)
            b0 = stage.tile([2, Lf], f32)
            nc.vector.tensor_tensor(b0[:], pinc[:], sf_[:], AL.subtract)
            b1 = stage.tile([2, Lf], f32)
            nc.vector.tensor_scalar(b1[:], b0[:],
                                    cst[0:2, 3 + 2 * bb:4 + 2 * bb],
                                    None, AL.add)
            rb_ = stage.tile([2, Lf], f32)
            nc.vector.tensor_scalar(rb_[:], b1[:], MAGIC, MAGIC,
                                    AL.add, AL.subtract)
            b2 = stage.tile([2, Lf], f32)
            nc.vector.scalar_tensor_tensor(b2[:], b1[:], 16.0, rb_[:],
                                           AL.add, AL.subtract)
            stgB = stage.tile([2, 512], bf16)
            nc.vector.tensor_copy(stgB[0:2, 0:250], b2[:, 0:250])
            nc.vector.tensor_tensor(stgB[0:2, 256:506], b2[:, 0:250],
                                    stgB[0:2, 0:250], AL.subtract)
            nc.sync.dma_start(sb_d[2 * bb:2 * bb + 2, :], stgB[:])
            bsrc = AP(tensor=sb_d[:].tensor, offset=bb * 1024,
                      ap=[[256, 2], [512, 2], [1, 250]])
            nc.sync.dma_start(lhs[37:39, bb * 500:(bb + 1) * 500], bsrc)
            bsrc2 = AP(tensor=sb_d[:].tensor, offset=bb * 1024,
                       ap=[[256, 2], [512, 2], [1, 250]])
            nc.scalar.dma_start(lhs[46:48, bb * 500:(bb + 1) * 500], bsrc2)

        def main_batch(bb):
            for n in (2 * bb, 2 * bb + 1):
                sn = snp.tile([125, 1920], f16)
                wv = wvp.tile([125, 1920], f16)
                for h in range(2):
                    col0 = n * 250 + h * 125
                    pv = ps_v.tile([125, 1024], f32)
                    pms = []
                    for cchunk in range(2):
                        s0 = cchunk * 480
                        nc.tensor.matmul(pv[:, cchunk * 512:cchunk * 512 + 480],
                                         lhs[32:48, col0:col0 + 125],
                                         rhsb[32:48, s0:s0 + 480],
                                         start=True, stop=True,
                                         tile_position=(32, 0))
                        pm = ps_m.tile([125, 512], f32)
                        nc.tensor.matmul(pm[:, 0:480],
                                         lhs[64:88, col0:col0 + 125],
                                         rhsb[64:88, s0:s0 + 480],
                                         start=True, stop=True,
                                         tile_position=(64, 0))
                        pms.append(pm)
                    pvv = pv[:].rearrange("p (b c) -> p b c", b=2)[:, :, 0:480]
                    snv = sn[:, h * 960:(h + 1) * 960].rearrange(
                        "p (b c) -> p b c", b=2)
                    nc.scalar.activation(snv, pvv, AF.Sin, bias=zbias[:, 0:1],
                                         scale=TWO_PI)
                    for cchunk in range(2):
                        q = h * 2 + cchunk
                        nc.vector.tensor_tensor(wv[:, q * 480:(q + 1) * 480],
                                                pms[cchunk][:, 0:480],
                                                sn[:, q * 480:(q + 1) * 480],
                                                AL.mult)
                for h in range(2):
                    oq = nc.sync if h == 0 else nc.gpsimd
                    oq.dma_start(out_d[n, h * 125:(h + 1) * 125, :],
                                 wv[:, h * 960:(h + 1) * 960])

        stage_batch(0)
        main_batch(0)
        stage_batch(1)
        main_batch(1)

    nc.compile()
    return nc


def _make_in_maps(inputs):
    x, phi, w_mag, b_mag, w_oct, b_oct = (inputs[k] for k in (
        "x", "phi", "w_mag", "b_mag", "w_oct", "b_oct"))
    rhs_base = _consts()  # [128, 960] bf16
    rhsb = np.zeros((128, SEG + 16), np.float32)
    rhsb[:, 0:SEG] = rhs_base.astype(np.float32)
    wm = w_mag[:, :, 0].astype(np.float32)  # [8, 256]
    for cc in range(2):
        rhsb[:, SEG + cc * 8:SEG + cc * 8 + 8] = wm[:, cc * 128:(cc + 1) * 128].T
    rhsb = _bf(rhsb)
    in_maps = []
    for c in range(NCORES):
        xs = np.ascontiguousarray(x[c * NPC:(c + 1) * NPC]).astype(np.float32)
        # cols: cc*1000 + n*250 + l
        xr = xs.reshape(NPC, 2, 128, 250).transpose(2, 1, 0, 3).reshape(128, 2000)
        xr = np.ascontiguousarray(xr)
        cst = np.zeros((128, 8), np.float32)
        cst[:, 0] = w_oct[0, 0:128, 0]
        cst[:, 1] = w_oct[0, 128:256, 0]
        cst[0:8, 2] = b_mag
        cst[0:2, 3] = phi[c * NPC:c * NPC + 2, 0, 0]
        cst[0:2, 5] = phi[c * NPC + 2:c * NPC + 4, 0, 0]
        cst[0, 4] = math.log(220.0) + math.log(2.0) * float(b_oct[0])
        in_maps.append(dict(xr=xr, xrb=_bf(xr), cst=cst, rhsb=rhsb,
                            ones2=_bf(np.ones((2, NPC * 250), np.float32))))
    return in_maps


def kernel(x, phi, w_mag, b_mag, w_oct, b_oct):
    from concourse.bass_utils import run_bass_kernel_spmd

    if "nc" not in _cache:
        _cache["nc"] = _build()
    nc = _cache["nc"]

    in_maps = _make_in_maps(dict(x=x, phi=phi, w_mag=w_mag, b_mag=b_mag,
                                 w_oct=w_oct, b_oct=b_oct))
    res = run_bass_kernel_spmd(nc, in_maps, core_ids=list(range(NCORES)))
    waves = [res.results[c]["wave"].astype(np.float32).reshape(NPC, 1, Lw)
             for c in range(NCORES)]
    return np.concatenate(waves, axis=0)
